# revision 20
# baseline (speedup 1.0000x reference)
"""Trainium2 Bass kernel for nn_ExcEmbedding (ragged caption/image cosine sims).

v3 design (baseline v1 ~114us, v2 ~125us):
  - bf16 streams + matmuls; f32 PSUM + epilogues.
  - All input DMAs enqueued up front; xin holds all 25 row tiles.
  - s1/s2 stats packed in one [64, 1024] PSUM tile per phase (rows 0-31 sum
    of y, rows 32-63 sum of y^2) -> 2 banks per phase.
  - ACT stays in the sqrt table set the whole kernel (dummy Sqrt preloads it;
    Lrelu/Square/Relu/Identity are in every set); one switch to the sigmoid
    set at the gate. rsqrt = DVE reciprocal_approx_fast(ACT Sqrt).
  - Elementwise split DVE/ACT only (no gpsimd: its queue must stay free for
    the collective machinery). DVE leaky = ts_mul(0.1) + tensor_tensor(max),
    cheaper than 1x-mode scalar_tensor_tensor.
  - Both criticals contain ONLY Pool-engine instructions, so no other engine
    queue is stalled by the inter-core launch skew (~50us across 8 cores):
    crit A (clears + broadcast descgen + kernel barrier + trigger) sits
    behind the img phase on the Pool queue; crit B (wait rsem>=16 + a tiny
    vv self-copy) gives the finals a RAW dep on the landed remote data.
  - The broadcast payload carries both V^T and (V^2)^T so nothing has to be
    squared on the receive side.
  - The final epilogue (rn, bias, rsqrt(Q) normalization) runs on the HOST:
    the device ships num/vg/q2 [32,256] and cv [32,1024] per core.
"""

import os
import numpy as np
import ml_dtypes

import concourse.bass as bass
import concourse.bacc as bacc
import concourse.mybir as mybir
import concourse.tile as tile
from concourse.bass_utils import run_bass_kernel_spmd

F32 = mybir.dt.float32
BF16 = mybir.dt.bfloat16
AF = mybir.ActivationFunctionType
ALU = mybir.AluOpType

NCORES = 8
B = 256
R = 36
T = 64
D = 1024
DSQ = 128
M = B // NCORES          # 32 local captions / images per core
NI = M * R // 128        # 9 img row tiles of (128, D)
NC = M * T // 128        # 16 cap row tiles of (128, D)
KD = D // 128            # 8 d-blocks
SEG = KD * M             # 256 columns per (rank, stat) block
SEG2 = 2 * SEG           # vt + vt2 per rank
CAP_PRE_TP = 3           # cap tiles issued before the v transposes

NOCOLL = os.environ.get("KV2_NOCOLL", "0") == "1"


def leaky_on_act(g):
    return g % 3 == 2


def square_on_dve(g):
    return g % 3 == 2 or g == 24


def build_program(beta: float):
    nc = bacc.Bacc("TRN2", target_bir_lowering=False, debug=False,
                   num_devices=NCORES)

    img_rows = nc.dram_tensor("img_rows", [NI * 128, D], BF16, kind="ExternalInput")
    cap_rows = nc.dram_tensor("cap_rows", [NC * 128, D], BF16, kind="ExternalInput")
    ei_t = nc.dram_tensor("ei_t", [128, NI * M], BF16, kind="ExternalInput")
    ec2_t = nc.dram_tensor("ec2_t", [128, NC * 2 * M], BF16, kind="ExternalInput")
    w_sq_t = nc.dram_tensor("w_sq_t", [128, D], BF16, kind="ExternalInput")
    w_ex_t = nc.dram_tensor("w_ex_t", [128, D], BF16, kind="ExternalInput")
    b_sq_t = nc.dram_tensor("b_sq_t", [DSQ, 1], F32, kind="ExternalInput")
    bexp_full = nc.dram_tensor("bexp_full", [128, SEG], F32, kind="ExternalInput")
    rlens = nc.dram_tensor("rlens", [M, 1], F32, kind="ExternalInput")
    idn32 = nc.dram_tensor("idn32", [M, M], F32, kind="ExternalInput")
    num_out = nc.dram_tensor("num_out", [M, B], F32, kind="ExternalOutput")
    vg_out = nc.dram_tensor("vg_out", [M, B], F32, kind="ExternalOutput")
    q2_out = nc.dram_tensor("q2_out", [M, B], F32, kind="ExternalOutput")
    cv_out = nc.dram_tensor("cv_out", [M, D], F32, kind="ExternalOutput")

    rsem = nc.alloc_semaphore(name="rsem")
    lsem = nc.alloc_semaphore(name="lsem")
    psem = nc.alloc_semaphore(name="psem")

    with tile.TileContext(nc) as tc:
        with (
            tc.tile_pool(name="consts", bufs=1) as consts,
            tc.tile_pool(name="xin", bufs=NI + NC + 2) as xin,
            tc.tile_pool(name="lt", bufs=4) as lt,
            tc.tile_pool(name="yp", bufs=6) as yp,
            tc.tile_pool(name="y2p", bufs=6) as y2p,
            tc.tile_pool(name="ep", bufs=1) as ep,
            tc.tile_pool(name="smalls", bufs=1) as smalls,
            tc.tile_pool(name="tsb", bufs=1) as tsb,
            tc.tile_pool(name="psA", bufs=2, space="PSUM") as psA,
            tc.tile_pool(name="psT", bufs=1, space="PSUM") as psT,
            tc.tile_pool(name="psF", bufs=1, space="PSUM") as psF,
        ):
            # ---- all input DMAs enqueued first (sync queue order) ----
            ei_sb = consts.tile([128, NI, M], BF16)
            nc.sync.dma_start(ei_sb[:], ei_t[:].rearrange("p (t c) -> p t c", t=NI))
            idn_sb = consts.tile([M, M], F32)
            nc.sync.dma_start(idn_sb[:], idn32[:])
            xs = []
            for t in range(NI):
                x = xin.tile([128, D], BF16, name="x")
                nc.sync.dma_start(x[:], img_rows[128 * t:128 * (t + 1), :])
                xs.append(x)
            ec_sb = consts.tile([128, NC, 2 * M], BF16)
            nc.sync.dma_start(ec_sb[:], ec2_t[:].rearrange("p (t c) -> p t c", t=NC))
            xcs = []
            for t in range(NC):
                xc = xin.tile([128, D], BF16, name="x")
                nc.sync.dma_start(xc[:], cap_rows[128 * t:128 * (t + 1), :])
                xcs.append(xc)
            wsq_sb = consts.tile([128, D], BF16)
            nc.sync.dma_start(wsq_sb[:], w_sq_t[:])
            wex_sb = consts.tile([128, D], BF16)
            nc.sync.dma_start(wex_sb[:], w_ex_t[:])
            bsq_sb = consts.tile([DSQ, 1], F32)
            nc.sync.dma_start(bsq_sb[:], b_sq_t[:])
            bexp_sb = consts.tile([128, SEG], F32)
            nc.sync.dma_start(bexp_sb[:], bexp_full[:])
            rlens_sb = consts.tile([M, 1], F32)
            nc.sync.dma_start(rlens_sb[:], rlens[:])

            # ---- dummy Sqrt pins the sqrt table set during DMA warmup ----
            dumm = smalls.tile([1, 1], F32, name="dumm")
            nc.vector.memset(dumm[:], 1.0)
            dum2 = smalls.tile([1, 1], F32, name="dum2")
            nc.scalar.activation(dum2[:], dumm[:], AF.Sqrt)

            def leaky_square(x, g):
                y = yp.tile([128, D], BF16, name="y")
                if leaky_on_act(g):
                    nc.scalar.activation(y[:], x[:], AF.Prelu, alpha=0.1)
                else:
                    xt = lt.tile([128, D], BF16, name="xt")
                    nc.vector.tensor_scalar_mul(xt[:], x[:], 0.1)
                    nc.vector.tensor_tensor(y[:], x[:], xt[:], op=ALU.max)
                y2 = y2p.tile([128, D], BF16, name="y2")
                if square_on_dve(g):
                    nc.vector.tensor_tensor(y2[:], y[:], y[:], op=ALU.mult)
                else:
                    nc.scalar.square(y2[:], y[:])
                return y, y2

            # ---- img phase ----
            s12i = psA.tile([2 * M, D], F32, tag="acc", name="s12i")
            for t in range(NI):
                y, y2 = leaky_square(xs[t], t)
                for h in range(2):
                    cs = slice(512 * h, 512 * (h + 1))
                    nc.tensor.matmul(s12i[0:M, cs], ei_sb[:, t, :], y[:, cs],
                                     start=(t == 0), stop=(t == NI - 1),
                                     skip_group_check=True)
                    nc.tensor.matmul(s12i[M:2 * M, cs], ei_sb[:, t, :], y2[:, cs],
                                     start=(t == 0), stop=(t == NI - 1),
                                     skip_group_check=True)

            # rsqrt with one Newton-Raphson pass: the ACT sqrt table is only
            # ~0.4% accurate (65536 ULP budget); r1 = r0*(1.5 - 0.5*x*r0^2)
            # squares that error away. Runs per 512-col half so transposes
            # can start after the first half.
            def rsqrt_half(s12_tile, cs, nm):
                sq = ep.tile([M, 512], F32, name=f"sq{nm}")
                nc.scalar.activation(sq[:], s12_tile[M:2 * M, cs], AF.Sqrt)
                r0 = ep.tile([M, 512], F32, name=f"r0{nm}")
                nc.vector.reciprocal_approx_fast(r0[:], sq[:])
                a = ep.tile([M, 512], F32, name=f"a{nm}")
                nc.vector.tensor_tensor(a[:], r0[:], r0[:], op=ALU.mult)
                b = ep.tile([M, 512], F32, name=f"b{nm}")
                nc.vector.scalar_tensor_tensor(b[:], s12_tile[M:2 * M, cs],
                                               -0.5, a[:],
                                               op0=ALU.mult, op1=ALU.mult)
                c = ep.tile([M, 512], F32, name=f"c{nm}")
                nc.vector.tensor_scalar_add(c[:], b[:], 1.5)
                r1 = ep.tile([M, 512], F32, name=f"r1{nm}")
                nc.vector.tensor_tensor(r1[:], r0[:], c[:], op=ALU.mult)
                return r1

            # ---- cap phase part 1 (keeps the PE busy during img epilogue) ----
            s12c = psA.tile([2 * M, D], F32, tag="acc", name="s12c")

            def cap_tile(t):
                yc, yc2 = leaky_square(xcs[t], NI + t)
                for h in range(2):
                    cs = slice(512 * h, 512 * (h + 1))
                    nc.tensor.matmul(s12c[0:M, cs], ec_sb[:, t, 0:M], yc[:, cs],
                                     start=(t == 0), stop=(t == NC - 1),
                                     skip_group_check=True)
                    nc.tensor.matmul(s12c[M:2 * M, cs], ec_sb[:, t, M:2 * M],
                                     yc2[:, cs],
                                     start=(t == 0), stop=(t == NC - 1),
                                     skip_group_check=True)

            # ---- img epilogue (by halves) + transpose v -> my_vb bf16 ----
            v = smalls.tile([M, D], F32, name="v")
            vps = psT.tile([128, SEG], F32, tag="t", name="vps")
            my_vb = tsb.tile([128, SEG], BF16, name="my_vb")
            cap_issued = 0
            for h in range(2):
                cs = slice(512 * h, 512 * (h + 1))
                while cap_issued < 2 * (h + 1):
                    cap_tile(cap_issued)
                    cap_issued += 1
                r1 = rsqrt_half(s12i, cs, f"i{h}")
                nc.vector.scalar_tensor_tensor(v[:, cs], s12i[0:M, cs],
                                               1.0 / R, r1[:],
                                               op0=ALU.mult, op1=ALU.mult)
                for k in range(4 * h, 4 * (h + 1)):
                    nc.tensor.transpose(vps[:, M * k:M * (k + 1)],
                                        v[:, 128 * k:128 * (k + 1)], idn_sb[:])
                nc.vector.tensor_copy(my_vb[:, 128 * h:128 * (h + 1)],
                                      vps[:, 128 * h:128 * (h + 1)])

            # ---- critical A: Pool-only, so no other engine stalls on the
            # inter-core barrier; it orders clears before any flight ----
            vv = tsb.tile([128, NCORES * SEG], BF16, name="vv")
            if NOCOLL:
                for g in range(NCORES):
                    nc.vector.tensor_copy(vv[:, SEG * g:SEG * (g + 1)],
                                          my_vb[:])
            else:
                with tc.tile_critical():
                    nc.gpsimd.sem_clear(rsem)
                    nc.gpsimd.sem_clear(lsem)
                    nc.gpsimd.sem_clear(psem)
                    rank = nc.gpsimd.partition_id()
                    nc.gpsimd.remote_dma_broadcast(
                        vv[:, bass.ds(rank * SEG, SEG)], my_vb[:],
                        remote_sem=rsem, local_sem=lsem,
                        rdests=[(0, j) for j in range(NCORES)],
                    ).then_inc(psem, 1)
                    nc.gpsimd.wait_ge(psem, 1)
                    nc.gpsimd.bir_kernel_barrier_wait([list(range(NCORES))])
                    nc.gpsimd.trigger_dma(count=1)

            # ---- cap phase part 2 ----
            for t in range(cap_issued, NC):
                cap_tile(t)

            # ---- cap epilogue (by halves) + transpose cv -> cvt bf16 ----
            cv = smalls.tile([M, D], F32, name="cv")
            cvps = psT.tile([128, SEG], F32, tag="t2", name="cvps")
            cvt = tsb.tile([128, SEG], BF16, name="cvt")
            for h in range(2):
                cs = slice(512 * h, 512 * (h + 1))
                r1 = rsqrt_half(s12c, cs, f"c{h}")
                nc.vector.scalar_tensor_tensor(cv[:, cs], s12c[0:M, cs],
                                               rlens_sb[:], r1[:],
                                               op0=ALU.mult, op1=ALU.mult)
                for k in range(4 * h, 4 * (h + 1)):
                    nc.tensor.transpose(cvps[:, M * k:M * (k + 1)],
                                        cv[:, 128 * k:128 * (k + 1)], idn_sb[:])
                nc.vector.tensor_copy(cvt[:, 128 * h:128 * (h + 1)],
                                      cvps[:, 128 * h:128 * (h + 1)])
            nc.sync.dma_start(cv_out[:], cv[:])

            # ---- gate ----
            ht_ps = psF.tile([DSQ, M], F32, tag="f", name="ht_ps")
            for k in range(KD):
                nc.tensor.matmul(ht_ps[:], wsq_sb[:, 128 * k:128 * (k + 1)],
                                 cvt[:, M * k:M * (k + 1)],
                                 start=(k == 0), stop=(k == KD - 1),
                                 skip_group_check=True)
            ht = tsb.tile([DSQ, M], BF16, name="ht")
            nc.scalar.activation(ht[:], ht_ps[:], AF.Relu, bias=bsq_sb[:])

            gps = psT.tile([128, SEG], F32, tag="g", name="gps")
            for k in range(KD):
                nc.tensor.matmul(gps[:, M * k:M * (k + 1)],
                                 wex_sb[:, 128 * k:128 * (k + 1)], ht[:],
                                 skip_group_check=True)
            gpb = tsb.tile([128, SEG], F32, name="gpb")
            nc.vector.tensor_tensor(gpb[:], gps[:], bexp_sb[:], op=ALU.add)
            gt = tsb.tile([128, SEG], BF16, name="gt")
            nc.scalar.activation(gt[:], gpb[:], AF.Sigmoid)
            g2t = tsb.tile([128, SEG], BF16, name="g2t")
            nc.vector.tensor_tensor(g2t[:], gt[:], gt[:], op=ALU.mult)
            at = tsb.tile([128, SEG], BF16, name="at")
            nc.vector.tensor_tensor(at[:], gt[:], cvt[:], op=ALU.mult)

            # ---- critical B: wait for the gathered payload; the tiny
            # self-copy writes vv so the finals get a RAW dep on it ----
            vtch = smalls.tile([1, 2], BF16, name="vtch")
            if not NOCOLL:
                with tc.tile_critical():
                    nc.gpsimd.wait_ge(rsem, NCORES * 2)
                    nc.gpsimd.tensor_copy(vtch[:], vv[0:1, 0:2])
                    nc.gpsimd.tensor_copy(vv[0:1, 0:2], vtch[:])

            # ---- finals: num/vg/q2 [M, B] ----
            vt2 = tsb.tile([128, NCORES * SEG], BF16, name="vt2")
            nc.vector.tensor_tensor(vt2[:], vv[:], vv[:], op=ALU.mult)
            vv4 = vv[:].rearrange("p (g k c) -> p g k c", g=NCORES, k=KD)
            vt24 = vt2[:].rearrange("p (g k c) -> p g k c", g=NCORES, k=KD)
            num_ps = psF.tile([M, B], F32, tag="f", name="num_ps")
            vg_ps = psF.tile([M, B], F32, tag="f", name="vg_ps")
            q2_ps = psF.tile([M, B], F32, tag="f", name="q2_ps")
            for k in range(KD):
                ks = slice(M * k, M * (k + 1))
                nc.tensor.matmul(num_ps[:], at[:, ks], vv4[:, :, k, :],
                                 start=(k == 0), stop=(k == KD - 1),
                                 skip_group_check=True)
                nc.tensor.matmul(vg_ps[:], gt[:, ks], vv4[:, :, k, :],
                                 start=(k == 0), stop=(k == KD - 1),
                                 skip_group_check=True)
                nc.tensor.matmul(q2_ps[:], g2t[:, ks], vt24[:, :, k, :],
                                 start=(k == 0), stop=(k == KD - 1),
                                 skip_group_check=True)

            # ---- ship raw stats; the normalization epilogue runs on host ----
            nsb = smalls.tile([M, B], F32, name="nsb")
            nc.vector.tensor_copy(nsb[:], num_ps[:])
            nc.sync.dma_start(num_out[:], nsb[:])
            vsb = smalls.tile([M, B], F32, name="vsb")
            nc.vector.tensor_copy(vsb[:], vg_ps[:])
            nc.sync.dma_start(vg_out[:], vsb[:])
            qsb = smalls.tile([M, B], F32, name="qsb")
            nc.vector.tensor_copy(qsb[:], q2_ps[:])
            nc.sync.dma_start(q2_out[:], qsb[:])

    nc.compile()
    return nc


_PROG_CACHE: dict = {}


def get_program(beta: float):
    if beta not in _PROG_CACHE:
        _PROG_CACHE[beta] = build_program(beta)
    return _PROG_CACHE[beta]


def make_in_maps(img_embed, cap_embed, lens, W_sq, b_sq, W_ex, b_ex):
    bf = ml_dtypes.bfloat16
    img_bf = np.ascontiguousarray(img_embed, dtype=np.float32).astype(bf)
    cap_bf = np.ascontiguousarray(cap_embed, dtype=np.float32).astype(bf)
    lens_i = np.asarray(lens).astype(np.int64)

    # W_sq (D, DSQ) -> [128, KD*128]: w_sq_t[p, 128k+j] = W_sq[128k+p, j]
    w_sq_np = np.asarray(W_sq, dtype=np.float32).astype(bf)
    w_sq_t_np = np.ascontiguousarray(
        w_sq_np.reshape(KD, 128, DSQ).transpose(1, 0, 2).reshape(128, D))
    w_ex_t_np = np.ascontiguousarray(np.asarray(W_ex, dtype=np.float32).astype(bf))
    b_sq_np = np.ascontiguousarray(
        np.asarray(b_sq, dtype=np.float32).reshape(DSQ, 1))
    # bexp_full[p, M*k + c] = +b_ex[128k + p]
    bex = np.asarray(b_ex, dtype=np.float32)
    bexp_np = np.ascontiguousarray(
        np.repeat(bex.reshape(KD, 128).T, M, axis=1).reshape(128, SEG))
    idn_np = np.eye(M, dtype=np.float32)

    ei_np = np.zeros((NI * 128, M), dtype=np.float32)
    rows_i = np.arange(M * R)
    ei_np[rows_i, rows_i // R] = 1.0
    ei_t_np = ei_np.reshape(NI, 128, M).transpose(1, 0, 2).reshape(
        128, NI * M).astype(bf)

    in_maps = []
    for j in range(NCORES):
        sl = slice(M * j, M * (j + 1))
        lens_local = lens_i[sl]
        ec2_np = np.zeros((M * T, 2 * M), dtype=np.float32)
        rows = np.arange(M * T)
        cidx = rows // T
        tidx = rows % T
        ec2_np[rows, M + cidx] = 1.0
        keep = tidx < lens_local[cidx]
        ec2_np[rows[keep], cidx[keep]] = 1.0
        ec2_t_np = ec2_np.reshape(NC, 128, 2 * M).transpose(1, 0, 2).reshape(
            128, NC * 2 * M).astype(bf)
        rlens_np = (1.0 / lens_local.astype(np.float64)).astype(
            np.float32).reshape(M, 1)

        in_maps.append({
            "img_rows": np.ascontiguousarray(img_bf[sl].reshape(M * R, D)),
            "cap_rows": np.ascontiguousarray(cap_bf[sl].reshape(M * T, D)),
            "ei_t": np.ascontiguousarray(ei_t_np),
            "ec2_t": np.ascontiguousarray(ec2_t_np),
            "w_sq_t": w_sq_t_np,
            "w_ex_t": w_ex_t_np,
            "b_sq_t": b_sq_np,
            "bexp_full": bexp_np,
            "rlens": rlens_np,
            "idn32": idn_np,
        })
    return in_maps


LAST_RESULT = None
EPS = 1e-8


def kernel(img_embed, cap_embed, lens, W_sq, b_sq, W_ex, b_ex, beta, beta1):
    global LAST_RESULT
    beta_f = float(np.asarray(beta).reshape(-1)[0])
    nc = get_program(beta_f)
    in_maps = make_in_maps(img_embed, cap_embed, lens, W_sq, b_sq, W_ex, b_ex)
    res = run_bass_kernel_spmd(nc, in_maps, core_ids=list(range(NCORES)))
    LAST_RESULT = res
    sims = np.empty((B, B), dtype=np.float32)
    for j in range(NCORES):
        r = res.results[j]
        num = r["num_out"].astype(np.float64)   # (M, B)
        vg = r["vg_out"].astype(np.float64)
        q2 = r["q2_out"].astype(np.float64)
        cv = r["cv_out"].astype(np.float64)     # (M, D)
        rn = 1.0 / (np.sqrt((cv * cv).sum(axis=1, keepdims=True)) + EPS)
        bias = beta_f * cv.sum(axis=1, keepdims=True) * rn
        denom = np.sqrt(q2 + 2.0 * beta_f * vg + beta_f * beta_f * D) + EPS
        simst = (num * rn + bias) / denom       # (M, B) = sims[:, block].T
        sims[:, M * j:M * (j + 1)] = simst.T.astype(np.float32)
    return sims
